# revision 1
# baseline (speedup 1.0000x reference)
"""MedianPool2d (3x3, stride 1, zero-pad 1) Trainium2 Bass kernel.

Full input x: (8, 64, 256, 256) fp32.  Sharding: pure data parallel over
batch -> core i processes x[i] (64, 256, 256).

Per-core layout: 128 SBUF partitions = (h, c) with p = h*64 + c, where
h in {0,1} picks the top/bottom 128-row half of the image and c the
channel.  Each partition processes a strip of HH=128 rows x 256 cols,
with a 1-row halo on each side (zero at the image border, neighbor rows
at the half boundary - both come in via DMA / memset).  Rows are stored
padded to WP=258 with zero columns at 0 and 257, so every tap of the
3x3 window is a pure free-dim offset.

Median of 9 = med3(max3(column mins), med3(column medians),
                   min3(column maxes))  -- exact, 15 min/max passes/pixel
with vertical row-pair sharing and horizontal even/odd pair sharing.
All elementwise work runs on the DVE (this toolchain rejects
TensorTensor on GPSIMD and CCE min/max accum on DMA); DMA is fully
overlapped by the Tile scheduler, and merge/final temporaries alias the
slots of dead earlier-stage buffers so R=16 chunks fit in SBUF.
"""

import numpy as np

B, C, H, W = 8, 64, 256, 256
NCORES = 8
HH = H // 2          # rows per half-strip
WP = W + 2           # padded row width

_CACHE = {}


def _build(R=8, gp_rows=0):
    """Build the Bass module for one core: x (64,256,256) f32 -> out same.

    gp_rows: number of output rows (of each chunk's R) computed on GPSIMD
    instead of the DVE.
    """
    import concourse.bacc as bacc
    import concourse.mybir as mybir
    from concourse.tile import TileContext

    MIN = mybir.AluOpType.min
    MAX = mybir.AluOpType.max
    f32 = mybir.dt.float32

    assert HH % R == 0
    assert 0 <= gp_rows < R
    K = HH // R                     # chunks per strip

    nc = bacc.Bacc("TRN2", name="median_pool2d")
    x = nc.dram_tensor("x", [C, H, W], f32, kind="ExternalInput")
    out = nc.dram_tensor("out", [C, H, W], f32, kind="ExternalOutput")

    xg = x.ap()                     # global view [c, 256, 256]
    og = out.ap()

    def tt(out_ap, in0, in1, op):
        """Elementwise tensor_tensor, row-split DVE/GPSIMD.

        All APs are [128, rows, width]; the row dim is axis 1.
        """
        rows = out_ap.shape[1]
        split = rows - gp_rows if rows > gp_rows else rows
        nc.vector.tensor_tensor(
            out=out_ap[:, 0:split], in0=in0[:, 0:split], in1=in1[:, 0:split],
            op=op,
        )
        if split < rows:
            nc.gpsimd.tensor_tensor(
                out=out_ap[:, split:rows], in0=in0[:, split:rows],
                in1=in1[:, split:rows], op=op,
            )

    with TileContext(nc) as tc:
        with (
            tc.tile_pool(name="io_in", bufs=3) as in_pool,
            tc.tile_pool(name="io_out", bufs=3) as out_pool,
            tc.tile_pool(name="vert", bufs=1) as v_pool,
            tc.tile_pool(name="merge", bufs=1) as m_pool,
        ):
            for k in range(K):
                r0 = k * R                      # first output row (half-local)
                # ---- load input chunk: rows r0-1 .. r0+R (R+2 rows) ----
                it = in_pool.tile([128, (R + 2) * WP], f32, name="it", tag="it")
                it3 = it.rearrange("p (r w) -> p r w", w=WP)
                # zero pad columns 0 and 257 for all rows
                nc.vector.memset(it3[:, :, 0:WP:WP - 1], 0.0)
                # top half: global rows r0-1 .. r0+R+1 (clip at k==0)
                if k == 0:
                    nc.vector.memset(it3[0:64, 0:1, 1:W + 1], 0.0)
                    nc.sync.dma_start(
                        out=it3[0:64, 1:R + 2, 1:W + 1],
                        in_=xg[:, 0:R + 1, :],
                    )
                else:
                    nc.sync.dma_start(
                        out=it3[0:64, :, 1:W + 1],
                        in_=xg[:, r0 - 1:r0 + R + 1, :],
                    )
                # bottom half: global rows HH+r0-1 .. HH+r0+R+1 (clip at last)
                if k == K - 1:
                    nc.vector.memset(it3[64:128, R + 1:R + 2, 1:W + 1], 0.0)
                    nc.sync.dma_start(
                        out=it3[64:128, 0:R + 1, 1:W + 1],
                        in_=xg[:, HH + r0 - 1:H, :],
                    )
                else:
                    nc.sync.dma_start(
                        out=it3[64:128, :, 1:W + 1],
                        in_=xg[:, HH + r0 - 1:HH + r0 + R + 1, :],
                    )

                # ---- vertical sort3 over rows (full padded width) ----
                X0 = it3[:, 0:R, :]
                X1 = it3[:, 1:R + 1, :]
                X2 = it3[:, 2:R + 2, :]

                def vtile(name):
                    t = v_pool.tile([128, R * WP], f32, name=name, tag=name)
                    return t.rearrange("p (r w) -> p r w", w=WP)

                P3 = vtile("bP")
                Q3 = vtile("bQ")
                Lo3 = vtile("bLo")
                W3 = vtile("bW")
                Me3 = vtile("bMe")
                Hi3 = vtile("bHi")

                tt(P3, X0, X1, MIN)
                tt(Q3, X0, X1, MAX)
                tt(Lo3, P3, X2, MIN)
                tt(W3, Q3, X2, MIN)
                tt(Me3, P3, W3, MAX)
                tt(Hi3, Q3, X2, MAX)

                # ---- horizontal merge (width 256 of 258) ----
                lo = [Lo3[:, :, d:d + W] for d in range(3)]
                me = [Me3[:, :, d:d + W] for d in range(3)]
                hi = [Hi3[:, :, d:d + W] for d in range(3)]

                def mtile(name):
                    t = m_pool.tile([128, R * W], f32, name=name, tag=name)
                    return t.rearrange("p (r w) -> p r w", w=W)

                mA = mtile("mA")
                mC = mtile("mC")
                mB = mtile("mB")
                mT = mtile("mT")
                mU = mtile("mU")
                mV = mtile("mV")

                # A = max3(lo)
                tt(mT, lo[0], lo[1], MAX)
                tt(mA, mT, lo[2], MAX)
                # C = min3(hi)
                tt(mU, hi[0], hi[1], MIN)
                tt(mC, mU, hi[2], MIN)
                # B = med3(me) = max(min(a,b), min(max(a,b), c))
                tt(mT, me[0], me[1], MIN)
                tt(mU, me[0], me[1], MAX)
                tt(mV, mU, me[2], MIN)
                tt(mB, mT, mV, MAX)

                # out = med3(A, B, C)
                ot = out_pool.tile([128, R * W], f32, name="ot", tag="ot")
                ot3 = ot.rearrange("p (r w) -> p r w", w=W)
                tt(mT, mA, mB, MIN)
                tt(mU, mA, mB, MAX)
                tt(mV, mU, mC, MIN)
                tt(ot3, mT, mV, MAX)

                # ---- store ----
                nc.sync.dma_start(out=og[:, r0:r0 + R, :], in_=ot3[0:64])
                nc.sync.dma_start(
                    out=og[:, HH + r0:HH + r0 + R, :], in_=ot3[64:128]
                )

    nc.compile()
    return nc


def _build_shared(R=8, gp_frac=0.0, dtype="float32", in_bufs=None, out_bufs=None):
    """15-op/pixel variant: vertical pair sharing + horizontal even/odd
    pair sharing in the merge.  gp_frac: fraction of rows of every
    elementwise op executed on GPSIMD instead of the DVE (unsupported by
    the current toolchain - keep 0).  dtype: compute dtype on-chip;
    float16 doubles DVE throughput on step-1 ops at ~2e-4 max rel err."""
    import concourse.bacc as bacc
    import concourse.mybir as mybir
    from concourse.tile import TileContext

    MIN = mybir.AluOpType.min
    MAX = mybir.AluOpType.max
    f32 = mybir.dt.float32
    cdt = getattr(mybir.dt, dtype)
    cast = cdt != f32

    assert HH % R == 0 and R % 2 == 0
    K = HH // R
    Rh = R // 2

    nc = bacc.Bacc("TRN2", name="median_pool2d_s")
    x = nc.dram_tensor("x", [C, H, W], f32, kind="ExternalInput")
    out = nc.dram_tensor("out", [C, H, W], f32, kind="ExternalOutput")
    xg = x.ap()
    og = out.ap()
    dma_io = nc.gpsimd if cast else nc.sync

    def tt(out_ap, in0, in1, op):
        rows = out_ap.shape[1]
        gp = int(rows * gp_frac + 0.5)
        split = rows - gp
        if split > 0:
            nc.vector.tensor_tensor(
                out=out_ap[:, 0:split], in0=in0[:, 0:split],
                in1=in1[:, 0:split], op=op,
            )
        if split < rows:
            nc.gpsimd.tensor_tensor(
                out=out_ap[:, split:rows], in0=in0[:, split:rows],
                in1=in1[:, split:rows], op=op,
            )

    if in_bufs is None:
        in_bufs = 3 if R <= 8 else 2
    if out_bufs is None:
        out_bufs = 3 if R <= 8 else 1
    with TileContext(nc) as tc:
        with (
            tc.tile_pool(name="io_in", bufs=in_bufs) as in_pool,
            tc.tile_pool(name="io_out", bufs=out_bufs) as out_pool,
            tc.tile_pool(name="work", bufs=1) as w_pool,
        ):
            def wtile(name, rows, width, tag=None):
                t = w_pool.tile([128, rows * width], cdt, name=name,
                                tag=tag or name)
                return t.rearrange("p (r w) -> p r w", w=width)

            for k in range(K):
                r0 = k * R
                it = in_pool.tile([128, (R + 2) * WP], cdt, name="it", tag="it")
                it3 = it.rearrange("p (r w) -> p r w", w=WP)
                nc.vector.memset(it3[:, :, 0:WP:WP - 1], 0.0)
                if k == 0:
                    nc.vector.memset(it3[0:64, 0:1, 1:W + 1], 0.0)
                    dma_io.dma_start(out=it3[0:64, 1:R + 2, 1:W + 1],
                                      in_=xg[:, 0:R + 1, :])
                else:
                    dma_io.dma_start(out=it3[0:64, :, 1:W + 1],
                                      in_=xg[:, r0 - 1:r0 + R + 1, :])
                if k == K - 1:
                    nc.vector.memset(it3[64:128, R + 1:R + 2, 1:W + 1], 0.0)
                    dma_io.dma_start(out=it3[64:128, 0:R + 1, 1:W + 1],
                                      in_=xg[:, HH + r0 - 1:H, :])
                else:
                    dma_io.dma_start(out=it3[64:128, :, 1:W + 1],
                                      in_=xg[:, HH + r0 - 1:HH + r0 + R + 1, :])

                # ---- vertical: shared pair sort ----
                # pairs over in-tile row pairs (2i+1, 2i+2), i = 0..R/2-1
                Pm = wtile("Pm", Rh, WP)
                PM = wtile("PM", Rh, WP)
                tt(Pm, it3[:, 1:R + 1:2, :], it3[:, 2:R + 2:2, :], MIN)
                tt(PM, it3[:, 1:R + 1:2, :], it3[:, 2:R + 2:2, :], MAX)

                Lo3 = wtile("Lo", R, WP)
                Me3 = wtile("Me", R, WP)
                Hi3 = wtile("Hi", R, WP)
                tE = wtile("tE", Rh, WP)
                tO = wtile("tO", Rh, WP)
                a_e = it3[:, 0:R:2, :]          # third element, even out rows
                a_o = it3[:, 3:R + 2:2, :]      # rows 3,5,..,R+1 (count R/2)
                # even out rows y=0,2,..  (pair index i=y/2)
                tt(Lo3[:, 0:R:2], a_e, Pm, MIN)
                tt(Hi3[:, 0:R:2], a_e, PM, MAX)
                tt(tE, a_e, PM, MIN)
                tt(Me3[:, 0:R:2], Pm, tE, MAX)
                # odd out rows y=1,3,..   (pair index i=(y-1)/2)
                tt(Lo3[:, 1:R:2], a_o, Pm, MIN)
                tt(Hi3[:, 1:R:2], a_o, PM, MAX)
                tt(tO, a_o, PM, MIN)
                tt(Me3[:, 1:R:2], Pm, tO, MAX)

                # ---- merge: horizontal shared pairs ----
                NP = W // 2 + 1                 # 129 pairs over padded width
                # Pm/PM/tE/tO are dead after the vertical completions;
                # alias their slots (Rh*WP = 2064 >= R*NP = 2064 elems).
                PA = wtile("PA", R, NP, tag="Pm")
                PC = wtile("PC", R, NP, tag="PM")
                Um = wtile("Um", R, NP, tag="tE")
                Vm = wtile("Vm", R, NP, tag="tO")
                # PA/PC (in Pm/PM slots) are dead once mA/mC are built;
                # rotate tBe/tBo through the same slots.
                tBe = wtile("tBe", R, W // 2, tag="Pm")
                tBo = wtile("tBo", R, W // 2, tag="PM")
                mA = wtile("mA", R, W)
                mB = wtile("mB", R, W)
                mC = wtile("mC", R, W)

                ev = slice(0, WP, 2)            # padded even cols (129)
                od = slice(1, WP, 2)            # padded odd cols (129)
                tt(PA, Lo3[:, :, ev], Lo3[:, :, od], MAX)
                tt(mA[:, :, 0:W:2], PA[:, :, 0:NP - 1], Lo3[:, :, 2:WP:2], MAX)
                tt(mA[:, :, 1:W:2], PA[:, :, 1:NP], Lo3[:, :, 1:WP - 2:2], MAX)

                tt(PC, Hi3[:, :, ev], Hi3[:, :, od], MIN)
                tt(mC[:, :, 0:W:2], PC[:, :, 0:NP - 1], Hi3[:, :, 2:WP:2], MIN)
                tt(mC[:, :, 1:W:2], PC[:, :, 1:NP], Hi3[:, :, 1:WP - 2:2], MIN)

                tt(Um, Me3[:, :, ev], Me3[:, :, od], MIN)
                tt(Vm, Me3[:, :, ev], Me3[:, :, od], MAX)
                tt(tBe, Me3[:, :, 2:WP:2], Vm[:, :, 0:NP - 1], MIN)
                tt(mB[:, :, 0:W:2], Um[:, :, 0:NP - 1], tBe, MAX)
                tt(tBo, Me3[:, :, 1:WP - 2:2], Vm[:, :, 1:NP], MIN)
                tt(mB[:, :, 1:W:2], Um[:, :, 1:NP], tBo, MAX)

                # ---- final med3(A, B, C) ----
                # Lo/Me/Hi are dead once the merge pairs+completions ran;
                # alias their slots (R*WP >= R*W).
                mT = wtile("mT", R, W, tag="Lo")
                mU = wtile("mU", R, W, tag="Me")
                mV = wtile("mV", R, W, tag="Hi")
                ot = out_pool.tile([128, R * W], cdt, name="ot", tag="ot")
                ot3 = ot.rearrange("p (r w) -> p r w", w=W)
                tt(mT, mA, mB, MIN)
                tt(mU, mA, mB, MAX)
                tt(mV, mU, mC, MIN)
                tt(ot3, mT, mV, MAX)

                dma_io.dma_start(out=og[:, r0:r0 + R, :], in_=ot3[0:64])
                dma_io.dma_start(out=og[:, HH + r0:HH + r0 + R, :],
                                  in_=ot3[64:128])

    nc.compile()
    return nc


def _build_copy():
    """Calibration kernel: pure DMA passthrough x -> out."""
    import concourse.bacc as bacc
    import concourse.mybir as mybir
    from concourse.tile import TileContext

    f32 = mybir.dt.float32
    nc = bacc.Bacc("TRN2", name="median_copy_cal")
    x = nc.dram_tensor("x", [C, H, W], f32, kind="ExternalInput")
    out = nc.dram_tensor("out", [C, H, W], f32, kind="ExternalOutput")
    xf = x.ap().rearrange("c h w -> (c h) w").rearrange(
        "(n p) w -> n p w", p=128)
    of = out.ap().rearrange("c h w -> (c h) w").rearrange(
        "(n p) w -> n p w", p=128)
    n = xf.shape[0]
    with TileContext(nc) as tc:
        with tc.tile_pool(name="io", bufs=4) as pool:
            for i in range(0, n, 8):
                t = pool.tile([128, 8 * W], f32, name="t", tag="t")
                t3 = t.rearrange("p (n w) -> p n w", w=W)
                nc.sync.dma_start(out=t3[:], in_=xf[i:i + 8].rearrange(
                    "n p w -> p n w"))
                nc.sync.dma_start(out=of[i:i + 8].rearrange("n p w -> p n w"),
                                  in_=t3[:])
    nc.compile()
    return nc


def _get_nc(R=8, gp_rows=0, shared=False, gp_frac=0.0, copy=False,
            dtype="float32", in_bufs=None, out_bufs=None):
    key = (R, gp_rows, shared, gp_frac, copy, dtype, in_bufs, out_bufs)
    if key not in _CACHE:
        if copy:
            _CACHE[key] = _build_copy()
        elif shared:
            _CACHE[key] = _build_shared(R=R, gp_frac=gp_frac, dtype=dtype,
                                        in_bufs=in_bufs, out_bufs=out_bufs)
        else:
            _CACHE[key] = _build(R=R, gp_rows=gp_rows)
    return _CACHE[key]


def kernel(x: np.ndarray) -> np.ndarray:
    """MedianPool2d(3x3, s=1, p=1) on 8 NeuronCores. Bit-exact vs fp32
    reference (pure min/max selection network, no arithmetic)."""
    from concourse.bass_utils import run_bass_kernel_spmd

    assert x.shape == (B, C, H, W), x.shape
    x = np.ascontiguousarray(x, dtype=np.float32)
    try:
        nc = _get_nc(shared=True, R=16)
    except Exception:
        # fall back to the simpler 18-op builder (also bit-exact)
        nc = _get_nc(R=8)
    in_maps = [{"x": x[i]} for i in range(NCORES)]
    res = run_bass_kernel_spmd(nc, in_maps, core_ids=list(range(NCORES)))
    return np.stack([r["out"] for r in res.results], axis=0)



# revision 2
# speedup vs baseline: 1.9521x; 1.9521x over previous
"""MedianPool2d (3x3, stride 1, zero-pad 1) Trainium2 Bass kernel.

Full input x: (8, 64, 256, 256) fp32.  Sharding: pure data parallel over
batch -> core i processes x[i] (64, 256, 256).

The kernel computes in bfloat16: the median is a pure selection network
(min/max only), and rounding to bf16 is monotone, so the result is
exactly bf16(median_fp32(x)) -- max rel err ~0.4%, far inside the 2e-2
gate -- while DVE tensor_tensor runs at 2 elem/cycle/lane instead of 1.

Layout (host-prepared, zero compute on device for padding/casting):
  Per core the input is rearranged to xp[c, 258, 260] bf16:
    row t = global row t-1 (rows 0 and 257 are the zero halo);
    within a row, columns are parity-split with pads baked in:
      idx 0       = 0 (left halo, plays B[-1])
      idx 1+k     = col 2k+1 (odd cols),  k = 0..127   ("B")
      idx 129     = 0 (unused)
      idx 130+k   = col 2k   (even cols), k = 0..127   ("A")
      idx 258     = 0 (right halo, plays A[128])
      idx 259     = 0 (unused)
  The 3-tap window of output col c maps to unit-stride slices:
    even c=2k:  {B[k-1], A[k], B[k]} = {V[0:128], V[130:258], V[1:129]}
    odd  c=2k+1:{A[k], B[k], A[k+1]} = {V[130:258], V[1:129], V[131:259]}
  so every tensor_tensor AP has innermost step 1 (keeps the 2x DVE mode;
  stride-2 APs would drop to 1x).

SBUF partitions: p = h*64 + ch, h in {0,1} = top/bottom 128-row half.
Per chunk of R output rows: vertical sliding sort3 with row-pair sharing
(5 ops/px), horizontal merge with even/odd pair sharing (6 ops/px),
final med3 (4 ops/px).  Even/odd completions are fused into single
instructions via zero-stride broadcast APs.  Output is written
parity-split bf16 ([0:128]=even cols, [128:256]=odd) and re-interleaved
+ upcast on the host.
"""

import numpy as np

B, C, H, W = 8, 64, 256, 256
NCORES = 8
HH = H // 2          # rows per half-strip
WL = 260             # parity-split padded row width
HP = H + 2           # padded row count

_CACHE = {}


def _bcast_parity(ap3, last=None):
    """[p, r, w] AP -> [p, r, (0,2), w]: broadcast over the parity dim."""
    import concourse.mybir as mybir
    from concourse.ap import AP

    dims = [list(d) for d in ap3.ap]
    if last is not None:
        dims[-1][1] = last
    new = dims[:-1] + [[0, 2]] + [dims[-1]]
    return AP(tensor=ap3.tensor, offset=ap3.offset,
              ap=mybir.VecI64Pair(new))


def _thirds(ap3, pstride):
    """[p, r, w] AP -> [p, r, (pstride,2), (1,128)]: the two parity
    third-element slices (j=0: cols 0..127, j=1: cols pstride..)."""
    import concourse.mybir as mybir
    from concourse.ap import AP

    dims = [list(d) for d in ap3.ap]
    dims[-1] = [1, 128]
    new = dims[:-1] + [[pstride, 2]] + [dims[-1]]
    return AP(tensor=ap3.tensor, offset=ap3.offset,
              ap=mybir.VecI64Pair(new))


def _build_bf16(R=32, in_bufs=2, out_bufs=2):
    """Bass module for one core: xp (64,258,260) bf16 -> out (64,256,256)
    bf16 parity-split."""
    import concourse.bacc as bacc
    import concourse.mybir as mybir
    from concourse.tile import TileContext

    MIN = mybir.AluOpType.min
    MAX = mybir.AluOpType.max
    bf16 = mybir.dt.bfloat16

    assert HH % R == 0 and R % 2 == 0
    K = HH // R
    Rh = R // 2

    nc = bacc.Bacc("TRN2", name="median_pool2d_bf16")
    x = nc.dram_tensor("x", [C, HP, WL], bf16, kind="ExternalInput")
    out = nc.dram_tensor("out", [C, H, W], bf16, kind="ExternalOutput")
    xg = x.ap()
    og = out.ap()

    with TileContext(nc) as tc:
        with (
            tc.tile_pool(name="io_in", bufs=in_bufs) as in_pool,
            tc.tile_pool(name="io_out", bufs=out_bufs) as out_pool,
            tc.tile_pool(name="work", bufs=1) as w_pool,
        ):
            def wtile(name, rows, width, tag=None):
                t = w_pool.tile([128, rows * width], bf16, name=name,
                                tag=tag or name)
                return t.rearrange("p (r w) -> p r w", w=width)

            for k in range(K):
                r0 = k * R
                it = in_pool.tile([128, (R + 2) * WL], bf16, name="it",
                                  tag="it")
                it3 = it.rearrange("p (r w) -> p r w", w=WL)
                # top half: xp rows r0 .. r0+R+1; bottom: +HH
                nc.sync.dma_start(out=it3[0:64],
                                  in_=xg[:, r0:r0 + R + 2, :])
                nc.sync.dma_start(out=it3[64:128],
                                  in_=xg[:, HH + r0:HH + r0 + R + 2, :])

                # ---- vertical sliding sort3 (rows), width WL ----
                Pm = wtile("Pm", Rh, WL)
                PM = wtile("PM", Rh, WL)
                nc.vector.tensor_tensor(out=Pm[:], in0=it3[:, 1:R + 1:2],
                                        in1=it3[:, 2:R + 2:2], op=MIN)
                nc.vector.tensor_tensor(out=PM[:], in0=it3[:, 1:R + 1:2],
                                        in1=it3[:, 2:R + 2:2], op=MAX)

                # thirds: out row 2i -> it3 row 2i; out row 2i+1 -> 2i+3
                tdims = [list(d) for d in it3[:].ap]  # [p,(260,R+2),(1,260)]
                thr = type(it3[:])(
                    tensor=it3[:].tensor, offset=it3[:].offset,
                    ap=mybir.VecI64Pair(
                        [tdims[0], [2 * WL, Rh], [3 * WL, 2], [1, WL]]))
                Pm_b = _bcast_parity(Pm[:])
                PM_b = _bcast_parity(PM[:])

                Lo = wtile("Lo", R, WL)
                Hi = wtile("Hi", R, WL)
                tQ = wtile("tQ", R, WL)
                Me = wtile("Me", R, WL)
                nc.vector.tensor_tensor(out=Lo[:], in0=thr, in1=Pm_b, op=MIN)
                nc.vector.tensor_tensor(out=Hi[:], in0=thr, in1=PM_b, op=MAX)
                nc.vector.tensor_tensor(out=tQ[:], in0=thr, in1=PM_b, op=MIN)
                nc.vector.tensor_tensor(out=Me[:], in0=Pm_b, in1=tQ[:],
                                        op=MAX)

                # ---- horizontal merge: A=max3(Lo), C=min3(Hi), B=med3(Me)
                def halves(V):
                    return V[:, :, 130:258], V[:, :, 1:129]

                PA = wtile("PA", R, 128, tag="tQ")
                mA = wtile("mA", R, 256)
                a1, b1 = halves(Lo)
                nc.vector.tensor_tensor(out=PA[:], in0=a1, in1=b1, op=MAX)
                nc.vector.tensor_tensor(out=mA[:], in0=_bcast_parity(PA[:]),
                                        in1=_thirds(Lo[:], 131), op=MAX)

                PC = wtile("PC", R, 128, tag="tQ")
                mC = wtile("mC", R, 256)
                a2, b2 = halves(Hi)
                nc.vector.tensor_tensor(out=PC[:], in0=a2, in1=b2, op=MIN)
                nc.vector.tensor_tensor(out=mC[:], in0=_bcast_parity(PC[:]),
                                        in1=_thirds(Hi[:], 131), op=MIN)

                Um = wtile("Um", R, 128, tag="Pm")
                Vm = wtile("Vm", R, 128, tag="PM")
                a3, b3 = halves(Me)
                nc.vector.tensor_tensor(out=Um[:], in0=a3, in1=b3, op=MIN)
                nc.vector.tensor_tensor(out=Vm[:], in0=a3, in1=b3, op=MAX)
                tB = wtile("tB", R, 256, tag="Lo")
                mB = wtile("mB", R, 256)
                nc.vector.tensor_tensor(out=tB[:], in0=_bcast_parity(Vm[:]),
                                        in1=_thirds(Me[:], 131), op=MIN)
                nc.vector.tensor_tensor(out=mB[:], in0=_bcast_parity(Um[:]),
                                        in1=tB[:], op=MAX)

                # ---- final med3(A, B, C) ----
                mT = wtile("mT", R, 256, tag="Hi")
                mU = wtile("mU", R, 256, tag="Me")
                mV = wtile("mV", R, 256, tag="Lo")
                ot = out_pool.tile([128, R * 256], bf16, name="ot", tag="ot")
                ot3 = ot.rearrange("p (r w) -> p r w", w=256)
                nc.vector.tensor_tensor(out=mT[:], in0=mA[:], in1=mB[:],
                                        op=MIN)
                nc.vector.tensor_tensor(out=mU[:], in0=mA[:], in1=mB[:],
                                        op=MAX)
                nc.vector.tensor_tensor(out=mV[:], in0=mU[:], in1=mC[:],
                                        op=MIN)
                nc.vector.tensor_tensor(out=ot3[:], in0=mT[:], in1=mV[:],
                                        op=MAX)

                nc.sync.dma_start(out=og[:, r0:r0 + R, :], in_=ot3[0:64])
                nc.sync.dma_start(out=og[:, HH + r0:HH + r0 + R, :],
                                  in_=ot3[64:128])

    nc.compile()
    return nc


def _get_nc(R=32, in_bufs=2, out_bufs=2):
    key = (R, in_bufs, out_bufs)
    if key not in _CACHE:
        _CACHE[key] = _build_bf16(R=R, in_bufs=in_bufs, out_bufs=out_bufs)
    return _CACHE[key]


def _prep_core(xi, bf16):
    """(64,256,256) f32 -> (64,258,260) bf16 parity-split padded."""
    xp = np.zeros((C, HP, WL), dtype=bf16)
    xb = xi.astype(bf16)
    xp[:, 1:H + 1, 1:129] = xb[:, :, 1::2]
    xp[:, 1:H + 1, 130:258] = xb[:, :, 0::2]
    return xp


def kernel(x: np.ndarray) -> np.ndarray:
    """MedianPool2d(3x3, s=1, p=1) on 8 NeuronCores, bf16 selection
    network (exact median of the bf16-rounded input)."""
    import ml_dtypes
    from concourse.bass_utils import run_bass_kernel_spmd

    bf16 = ml_dtypes.bfloat16
    assert x.shape == (B, C, H, W), x.shape
    x = np.ascontiguousarray(x, dtype=np.float32)
    nc = _get_nc()
    in_maps = [{"x": _prep_core(x[i], bf16)} for i in range(NCORES)]
    res = run_bass_kernel_spmd(nc, in_maps, core_ids=list(range(NCORES)))
    y = np.empty((B, C, H, W), dtype=np.float32)
    for i in range(NCORES):
        o = res.results[i]["out"]
        y[i, :, :, 0::2] = o[:, :, 0:128]
        y[i, :, :, 1::2] = o[:, :, 128:256]
    return y


# revision 7
# speedup vs baseline: 2.0015x; 1.0253x over previous
"""MedianPool2d (3x3, stride 1, zero-pad 1) Trainium2 Bass kernel.

Full input x: (8, 64, 256, 256) fp32.  Sharding: pure data parallel over
batch -> core i processes x[i] (64, 256, 256).

The kernel computes in bfloat16: the median is a pure selection network
(min/max only), and rounding to bf16 is monotone, so the result is
exactly bf16(median_fp32(x)) -- max rel err ~0.4%, far inside the 2e-2
gate -- while DVE tensor_tensor runs at 2 elem/cycle/lane instead of 1.

Layout (host-prepared, zero compute on device for padding/casting):
  Per core the input is rearranged to xp[c, 258, 260] bf16:
    row t = global row t-1 (rows 0 and 257 are the zero halo);
    within a row, columns are parity-split with pads baked in:
      idx 0       = 0 (left halo, plays B[-1])
      idx 1+k     = col 2k+1 (odd cols),  k = 0..127   ("B")
      idx 129     = 0 (unused)
      idx 130+k   = col 2k   (even cols), k = 0..127   ("A")
      idx 258     = 0 (right halo, plays A[128])
      idx 259     = 0 (unused)
  The 3-tap window of output col c maps to unit-stride slices:
    even c=2k:  {B[k-1], A[k], B[k]} = {V[0:128], V[130:258], V[1:129]}
    odd  c=2k+1:{A[k], B[k], A[k+1]} = {V[130:258], V[1:129], V[131:259]}
  so every tensor_tensor AP has innermost step 1 (keeps the 2x DVE mode;
  stride-2 APs would drop to 1x).

SBUF partitions: p = h*64 + ch, h in {0,1} = top/bottom 128-row half.
Per chunk of R output rows: vertical sliding sort3 with row-pair sharing
(5 ops/px), horizontal merge with even/odd pair sharing (6 ops/px),
final med3 (4 ops/px).  Even/odd completions are fused into single
instructions via zero-stride broadcast APs.  Output is written
parity-split bf16 ([0:128]=even cols, [128:256]=odd) and re-interleaved
+ upcast on the host.
"""

import numpy as np

B, C, H, W = 8, 64, 256, 256
NCORES = 8
HH = H // 2          # rows per half-strip
WL = 260             # parity-split padded row width
HP = H + 2           # padded row count

_CACHE = {}


def _bcast_parity(ap3, last=None):
    """[p, r, w] AP -> [p, r, (0,2), w]: broadcast over the parity dim."""
    import concourse.mybir as mybir
    from concourse.ap import AP

    dims = [list(d) for d in ap3.ap]
    if last is not None:
        dims[-1][1] = last
    new = dims[:-1] + [[0, 2]] + [dims[-1]]
    return AP(tensor=ap3.tensor, offset=ap3.offset,
              ap=mybir.VecI64Pair(new))


def _thirds(ap3, pstride):
    """[p, r, w] AP -> [p, r, (pstride,2), (1,128)]: the two parity
    third-element slices (j=0: cols 0..127, j=1: cols pstride..)."""
    import concourse.mybir as mybir
    from concourse.ap import AP

    dims = [list(d) for d in ap3.ap]
    dims[-1] = [1, 128]
    new = dims[:-1] + [[pstride, 2]] + [dims[-1]]
    return AP(tensor=ap3.tensor, offset=ap3.offset,
              ap=mybir.VecI64Pair(new))


def _build_bf16(R=32, in_bufs=2, out_bufs=2, taper=0):
    """Bass module for one core: xp (64,258,260) bf16 -> out (64,256,256)
    bf16 parity-split.  taper>0 splits the first/last chunk into
    taper-row pieces so the pipeline fills/drains faster."""
    import concourse.bacc as bacc
    import concourse.mybir as mybir
    from concourse.tile import TileContext

    MIN = mybir.AluOpType.min
    MAX = mybir.AluOpType.max
    bf16 = mybir.dt.bfloat16

    if isinstance(R, (tuple, list)):
        chunks = list(R)
    elif taper:
        assert taper % 2 == 0 and R % taper == 0
        chunks = ([taper] * (R // taper) + [R] * (HH // R - 2)
                  + [taper] * (R // taper))
    else:
        chunks = [R] * (HH // R)
    assert sum(chunks) == HH and all(r % 2 == 0 for r in chunks)

    nc = bacc.Bacc("TRN2", name="median_pool2d_bf16")
    x = nc.dram_tensor("x", [C, HP, WL], bf16, kind="ExternalInput")
    out = nc.dram_tensor("out", [C, H, W], bf16, kind="ExternalOutput")
    xg = x.ap()
    og = out.ap()

    with TileContext(nc) as tc:
        with (
            tc.tile_pool(name="io_in", bufs=in_bufs) as in_pool,
            tc.tile_pool(name="io_out", bufs=out_bufs) as out_pool,
            tc.tile_pool(name="work", bufs=1) as w_pool,
        ):
            def wtile(name, rows, width, tag=None):
                t = w_pool.tile([128, rows * width], bf16, name=name,
                                tag=tag or name)
                return t.rearrange("p (r w) -> p r w", w=width)

            r0 = 0
            for R in chunks:
                Rh = R // 2
                it = in_pool.tile([128, (R + 2) * WL], bf16, name="it",
                                  tag="it")
                it3 = it.rearrange("p (r w) -> p r w", w=WL)
                # top half: xp rows r0 .. r0+R+1; bottom: +HH
                # (separate HWDGE queues so the two loads run in parallel)
                nc.sync.dma_start(out=it3[0:64],
                                  in_=xg[:, r0:r0 + R + 2, :])
                nc.scalar.dma_start(out=it3[64:128],
                                    in_=xg[:, HH + r0:HH + r0 + R + 2, :])

                # ---- vertical sliding sort3 (rows); only cols 0..258 are
                # consumed by the merge ----
                WV = WL - 1
                Pm = wtile("Pm", Rh, WL)
                PM = wtile("PM", Rh, WL)
                nc.vector.tensor_tensor(out=Pm[:, :, 0:WV],
                                        in0=it3[:, 1:R + 1:2, 0:WV],
                                        in1=it3[:, 2:R + 2:2, 0:WV], op=MIN)
                nc.vector.tensor_tensor(out=PM[:, :, 0:WV],
                                        in0=it3[:, 1:R + 1:2, 0:WV],
                                        in1=it3[:, 2:R + 2:2, 0:WV], op=MAX)

                # thirds: out row 2i -> it3 row 2i; out row 2i+1 -> 2i+3
                tdims = [list(d) for d in it3[:].ap]  # [p,(260,R+2),(1,260)]
                thr = type(it3[:])(
                    tensor=it3[:].tensor, offset=it3[:].offset,
                    ap=mybir.VecI64Pair(
                        [tdims[0], [2 * WL, Rh], [3 * WL, 2], [1, WV]]))
                Pm_b = _bcast_parity(Pm[:], last=WV)
                PM_b = _bcast_parity(PM[:], last=WV)

                Lo = wtile("Lo", R, WL)
                Hi = wtile("Hi", R, WL)
                tQ = wtile("tQ", R, WL)
                Me = wtile("Me", R, WL)
                nc.vector.tensor_tensor(out=Lo[:, :, 0:WV], in0=thr,
                                        in1=Pm_b, op=MIN)
                nc.vector.tensor_tensor(out=Hi[:, :, 0:WV], in0=thr,
                                        in1=PM_b, op=MAX)
                nc.vector.tensor_tensor(out=tQ[:, :, 0:WV], in0=thr,
                                        in1=PM_b, op=MIN)
                nc.vector.tensor_tensor(out=Me[:, :, 0:WV], in0=Pm_b,
                                        in1=tQ[:, :, 0:WV], op=MAX)

                # ---- horizontal merge: A=max3(Lo), C=min3(Hi), B=med3(Me)
                def halves(V):
                    return V[:, :, 130:258], V[:, :, 1:129]

                PA = wtile("PA", R, 128, tag="tQ")
                mA = wtile("mA", R, 256)
                a1, b1 = halves(Lo)
                nc.vector.tensor_tensor(out=PA[:], in0=a1, in1=b1, op=MAX)
                nc.vector.tensor_tensor(out=mA[:], in0=_bcast_parity(PA[:]),
                                        in1=_thirds(Lo[:], 131), op=MAX)

                PC = wtile("PC", R, 128, tag="tQ")
                mC = wtile("mC", R, 256)
                a2, b2 = halves(Hi)
                nc.vector.tensor_tensor(out=PC[:], in0=a2, in1=b2, op=MIN)
                nc.vector.tensor_tensor(out=mC[:], in0=_bcast_parity(PC[:]),
                                        in1=_thirds(Hi[:], 131), op=MIN)

                Um = wtile("Um", R, 128, tag="Pm")
                Vm = wtile("Vm", R, 128, tag="PM")
                a3, b3 = halves(Me)
                nc.vector.tensor_tensor(out=Um[:], in0=a3, in1=b3, op=MIN)
                nc.vector.tensor_tensor(out=Vm[:], in0=a3, in1=b3, op=MAX)
                tB = wtile("tB", R, 256, tag="Lo")
                mB = wtile("mB", R, 256)
                nc.vector.tensor_tensor(out=tB[:], in0=_bcast_parity(Vm[:]),
                                        in1=_thirds(Me[:], 131), op=MIN)
                nc.vector.tensor_tensor(out=mB[:], in0=_bcast_parity(Um[:]),
                                        in1=tB[:], op=MAX)

                # ---- final med3(A, B, C) ----
                mT = wtile("mT", R, 256, tag="Hi")
                mU = wtile("mU", R, 256, tag="Me")
                mV = wtile("mV", R, 256, tag="Lo")
                ot = out_pool.tile([128, R * 256], bf16, name="ot", tag="ot")
                ot3 = ot.rearrange("p (r w) -> p r w", w=256)
                nc.vector.tensor_tensor(out=mT[:], in0=mA[:], in1=mB[:],
                                        op=MIN)
                nc.vector.tensor_tensor(out=mU[:], in0=mA[:], in1=mB[:],
                                        op=MAX)
                nc.vector.tensor_tensor(out=mV[:], in0=mU[:], in1=mC[:],
                                        op=MIN)
                nc.vector.tensor_tensor(out=ot3[:], in0=mT[:], in1=mV[:],
                                        op=MAX)

                nc.scalar.dma_start(out=og[:, r0:r0 + R, :], in_=ot3[0:64])
                nc.scalar.dma_start(out=og[:, HH + r0:HH + r0 + R, :],
                                    in_=ot3[64:128])
                r0 += R

    nc.compile()
    return nc


def _get_nc(R=(8, 16, 32, 32, 32, 8), in_bufs=2, out_bufs=2, taper=0):
    key = (tuple(R) if isinstance(R, (tuple, list)) else R,
           in_bufs, out_bufs, taper)
    if key not in _CACHE:
        _CACHE[key] = _build_bf16(R=R, in_bufs=in_bufs, out_bufs=out_bufs,
                                  taper=taper)
    return _CACHE[key]


def _prep_core(xi, bf16):
    """(64,256,256) f32 -> (64,258,260) bf16 parity-split padded."""
    xp = np.zeros((C, HP, WL), dtype=bf16)
    xb = xi.astype(bf16)
    xp[:, 1:H + 1, 1:129] = xb[:, :, 1::2]
    xp[:, 1:H + 1, 130:258] = xb[:, :, 0::2]
    return xp


def kernel(x: np.ndarray) -> np.ndarray:
    """MedianPool2d(3x3, s=1, p=1) on 8 NeuronCores, bf16 selection
    network (exact median of the bf16-rounded input)."""
    import ml_dtypes
    from concourse.bass_utils import run_bass_kernel_spmd

    bf16 = ml_dtypes.bfloat16
    assert x.shape == (B, C, H, W), x.shape
    x = np.ascontiguousarray(x, dtype=np.float32)
    nc = _get_nc()
    in_maps = [{"x": _prep_core(x[i], bf16)} for i in range(NCORES)]
    res = run_bass_kernel_spmd(nc, in_maps, core_ids=list(range(NCORES)))
    y = np.empty((B, C, H, W), dtype=np.float32)
    for i in range(NCORES):
        o = res.results[i]["out"]
        y[i, :, :, 0::2] = o[:, :, 0:128]
        y[i, :, :, 1::2] = o[:, :, 128:256]
    return y
